# revision 3
# baseline (speedup 1.0000x reference)
"""Trainium2 Bass kernel: basic GCN layer, row-parallel over 8 NeuronCores.

    Y = relu( D^-1/2 (A + I) D^-1/2 (H @ W.T + b) ),  D = (A + I).sum(axis=1)

Sharding: core i owns output rows [i*1024, (i+1)*1024).  Each core receives
(A + I)[rows, :].T pre-tiled host-side into [128, 64*1024] fp8 so every DMA
descriptor moves an 8 KiB contiguous line (A is 0/1/2 — fp8 is lossless).
A stays fp8 in SBUF; matmuls use it as the moving operand against bf16
stationaries.

Schedule per core:
  - t=0: a tiny 2-rank dummy AllGather absorbs the one-time collective
    setup (entry barrier + ncfw startup) concurrently with the A load.
  - A loads in 8 chunks on two queues; the PE runs the row-sum matmuls
    (ones^T @ A-tile) paced with the arriving chunks.
  - At DMA end the row sums go out in ONE 8-rank AllGather (4 KiB).
  - The AG transport window is filled with the H @ W.T + b precompute
    (H.T arrives after A by queue order) and a PE warm-keeper chain.
  - When the AG lands: transpose row sums to [128, 64], dinv = rsqrt,
    scale the HW tiles by dinv columns, then the main matmuls
    (X^T A-tile into two PSUM halves), epilogue relu(dl * main), DMA out.
"""

import os
import sys

import numpy as np

for _p in ("/opt/trn_rl_repo", "/root/.axon_site/_ro/trn_rl_repo"):
    if _p not in sys.path and os.path.isdir(_p):
        sys.path.insert(0, _p)

N = 8192        # nodes
NCORES = 8
RPC = N // NCORES  # rows per core (1024)
P = 128         # partitions / tile edge
F = 128         # feature dim (in == out)


def _build_nc(n=8192, rpc=1024, f=128, ncores=8, warm1=40):
    import concourse.bass as bass  # noqa: F401
    import concourse.mybir as mybir
    from concourse import bacc, tile
    from concourse.masks import make_identity

    dt = mybir.dt
    f32, bf, f8 = dt.float32, dt.bfloat16, dt.float8e4

    P = 128
    kt = n // P                 # contraction tiles (64)
    NCH = 8                     # A DMA chunks
    kpc = kt // NCH             # k-tiles per chunk (8)
    RC = 512                    # PSUM half width
    NRC = rpc // RC             # 2 halves

    nc = bacc.Bacc("TRN2", num_devices=ncores)

    at = nc.dram_tensor("at", [P, kt * rpc], f8, kind="ExternalInput")   # (A+I)[rows].T pre-tiled
    ht = nc.dram_tensor("ht", [f, n], bf, kind="ExternalInput")          # H.T
    wt = nc.dram_tensor("wt", [f, f], bf, kind="ExternalInput")          # W.T
    bias = nc.dram_tensor("bias", [1, f], bf, kind="ExternalInput")      # b
    out = nc.dram_tensor("out", [f, rpc], f32, kind="ExternalOutput")    # Y[rows].T

    with tile.TileContext(nc) as tc:
        with (
            tc.tile_pool(name="const", bufs=1) as cpool,
            tc.tile_pool(name="abuf", bufs=1) as apool,
            tc.tile_pool(name="xbuf", bufs=1) as xpool,
            tc.tile_pool(name="work", bufs=1) as wpool,
            tc.tile_pool(name="pshw", bufs=2, space="PSUM") as pshw,
            tc.tile_pool(name="psbig", bufs=1, space="PSUM") as psbig,
            tc.tile_pool(name="dram", bufs=1, space="DRAM") as dpool,
        ):
            # ---- constants / small inputs ----
            wt_sb = cpool.tile([f, f], bf, tag="wt", name="wt_sb")
            bias_sb = cpool.tile([1, f], bf, tag="bias", name="bias_sb")
            ones_c = cpool.tile([P, 1], bf, tag="onesc", name="ones_c")
            ones_r = cpool.tile([1, 512], bf, tag="onesr", name="ones_r")
            ident = cpool.tile([P, P], f32, tag="ident", name="ident")
            dsrc = cpool.tile([1, 64], f32, tag="dsrc", name="dsrc")
            nc.vector.memset(ones_c[:], 1.0)
            nc.vector.memset(ones_r[:], 1.0)
            nc.vector.memset(dsrc[:], 1.0)
            make_identity(nc, ident[:])
            nc.scalar.dma_start(wt_sb[:], wt[:])
            nc.scalar.dma_start(bias_sb[:], bias[:])

            # ---- dummy collective: absorb barrier + ncfw startup ----
            ci_d = dpool.tile([1, 64], f32, tag="cid", name="ci_d")
            co_d = dpool.tile([2, 64], f32, tag="cod", name="co_d")
            nc.scalar.dma_start(ci_d[:], dsrc[:])
            nc.gpsimd.collective_compute(
                "AllGather", mybir.AluOpType.bypass,
                replica_groups=[[2 * i, 2 * i + 1] for i in range(ncores // 2)],
                ins=[ci_d.opt()], outs=[co_d.opt()],
            )

            # ---- A chunks on gpsimd+sync queues (A first, then H.T) ----
            a_ch = []
            for c in range(NCH):
                a_c = apool.tile([P, kpc * rpc], f8, tag=f"a{c}", name=f"a{c}")
                eng = nc.gpsimd if c % 2 == 0 else nc.sync
                eng.dma_start(a_c[:], at[:, c * kpc * rpc:(c + 1) * kpc * rpc])
                a_ch.append(a_c)

            ht_ch = []
            for c in range(NCH):
                h_c = cpool.tile([f, rpc], bf, tag=f"h{c}", name=f"ht{c}")
                eng = nc.gpsimd if c % 2 == 0 else nc.sync
                eng.dma_start(h_c[:], ht[:, c * rpc:(c + 1) * rpc])
                ht_ch.append(h_c)

            def a_slice(k, h):
                return a_ch[k // kpc][:, (k % kpc) * rpc + h * RC:
                                      (k % kpc) * rpc + (h + 1) * RC]

            # ---- row sums, paced with the A chunks ----
            ps_rs = [psbig.tile([1, RC], f32, tag=f"rs{h}", name=f"rs{h}")
                     for h in range(NRC)]
            for c in range(NCH):
                for kl in range(kpc):
                    k = c * kpc + kl
                    for h in range(NRC):
                        nc.tensor.matmul(ps_rs[h][0:1, :], ones_c[:, 0:1],
                                         a_slice(k, h),
                                         start=(k == 0), stop=(k == kt - 1))

            # row sums -> SBUF -> DRAM -> the one real AllGather
            rs_sb = wpool.tile([1, rpc], f32, tag="rs_sb", name="rs_sb")
            for h in range(NRC):
                nc.vector.tensor_copy(rs_sb[0:1, h * RC:(h + 1) * RC],
                                      ps_rs[h][0:1, :])
            ci = dpool.tile([1, rpc], f32, tag="ccin", name="cc_in")
            co = dpool.tile([ncores, rpc], f32, tag="ccout", name="cc_out",
                            addr_space="Shared")
            nc.sync.dma_start(ci[:], rs_sb[:])
            nc.gpsimd.collective_compute(
                "AllGather", mybir.AluOpType.bypass,
                replica_groups=[list(range(ncores))],
                ins=[ci.opt()], outs=[co.opt()],
            )

            # ---- HW = H @ W.T + b (fills the AG transport window) ----
            hw_sb = []
            for k in range(kt):
                ps_hw = pshw.tile([P, f], f32, tag="hw", name=f"hw{k}")
                nc.tensor.matmul(ps_hw[:, :],
                                 ht_ch[k // kpc][:, (k % kpc) * P:
                                                 (k % kpc + 1) * P],
                                 wt_sb[:, :], start=True, stop=False)
                nc.tensor.matmul(ps_hw[:, :], ones_r[0:1, 0:P],
                                 bias_sb[0:1, :], start=False, stop=True)
                hw_k = xpool.tile([P, f], bf, tag=f"hw_nb{k}", name=f"hw_nb{k}")
                nc.vector.tensor_copy(hw_k[:, :], ps_hw[:, :])
                hw_sb.append(hw_k)

            # ---- PE warm-keeper across the AG ----
            ps_warm = pshw.tile([1, 512], f32, tag="hw", name="ps_warm")
            for _ in range(warm1):
                nc.tensor.matmul(ps_warm[0:1, :], ones_r[0:1, 0:1],
                                 ones_r[0:1, :], start=True, stop=True)

            # ---- dl broadcast for the epilogue: dlb[p, i] = D_i^-1/2 ----
            dlb = wpool.tile([P, rpc], f32, tag="dlb", name="dlb")
            nc.gpsimd.dma_start(
                dlb[:].rearrange("p (o r) -> p o r", o=1),
                ci[0:1, :].partition_broadcast(P),
            )
            nc.scalar.sqrt(dlb[:, :], dlb[:, :])
            nc.vector.reciprocal(dlb[:, :], dlb[:, :])

            # ---- gathered sums -> dinv[p, k] = 1/sqrt(s[128k + p]) ----
            rs2d = wpool.tile([kt, P], f32, tag="rs2d", name="rs2d")
            nc.sync.dma_start(
                rs2d[:], co[:].rearrange("g (m p) -> (g m) p", p=P))
            ps_t = pshw.tile([P, kt], f32, tag="hw", name="ps_t")
            nc.tensor.transpose(ps_t[:, :], rs2d[:, :], ident[0:kt, 0:kt])
            dinv = wpool.tile([P, kt], f32, tag="dinv", name="dinv")
            nc.scalar.sqrt(dinv[:, :], ps_t[:, :])
            nc.vector.reciprocal(dinv[:, :], dinv[:, :])

            # ---- main matmuls: ps_main[h] += (dinv_k * HW_k)^T @ A_k ----
            ps_main = [psbig.tile([f, RC], f32, tag=f"main{h}", name=f"main{h}")
                       for h in range(NRC)]
            for k in range(kt):
                nc.vector.tensor_scalar_mul(hw_sb[k][:, :], hw_sb[k][:, :],
                                            dinv[:, k:k + 1])
                for h in range(NRC):
                    nc.tensor.matmul(
                        ps_main[h][:, :], hw_sb[k][:, :], a_slice(k, h),
                        start=(k == 0), stop=(k == kt - 1),
                    )

            # ---- epilogue: Y.T = relu(dl * main), DMA out per half ----
            y_sb = wpool.tile([f, rpc], f32, tag="y", name="y_sb")
            for h in range(NRC):
                sl = y_sb[:, h * RC:(h + 1) * RC]
                nc.vector.tensor_mul(sl, ps_main[h][:, :],
                                     dlb[:, h * RC:(h + 1) * RC])
                nc.vector.tensor_scalar_max(sl, sl, 0.0)
                nc.sync.dma_start(out[:, h * RC:(h + 1) * RC], sl)

    nc.compile()
    return nc


_CACHE = {}


def _get_nc():
    if "nc" not in _CACHE:
        _CACHE["nc"] = _build_nc()
    return _CACHE["nc"]


def _prep_in_maps(H, A, W, b):
    import ml_dtypes

    bf16 = ml_dtypes.bfloat16
    H = np.asarray(H, dtype=np.float32)
    A = np.asarray(A, dtype=np.float32)
    W = np.asarray(W, dtype=np.float32)
    b = np.asarray(b, dtype=np.float32)
    ht = np.ascontiguousarray(H.T.astype(bf16))
    wt = np.ascontiguousarray(W.T.astype(bf16))
    bias = np.ascontiguousarray(b.reshape(1, -1).astype(bf16))
    idx = np.arange(RPC)
    maps = []
    for i in range(NCORES):
        rows = slice(i * RPC, (i + 1) * RPC)
        Asl = A[rows, :].copy()
        Asl[idx, i * RPC + idx] += 1.0          # fold in A + I (0/1/2: exact)
        # pre-tile (A+I)[rows].T -> [128, kt*rpc] with 8 KiB-contiguous lines
        at = Asl.T.reshape(N // P, P, RPC).transpose(1, 0, 2).reshape(P, -1)
        maps.append({
            "at": np.ascontiguousarray(at.astype(ml_dtypes.float8_e4m3)),
            "ht": ht,
            "wt": wt,
            "bias": bias,
        })
    return maps


def run(H, A, W, b, trace=False):
    from concourse import bass_utils

    nc = _get_nc()
    res = bass_utils.run_bass_kernel_spmd(
        nc, _prep_in_maps(H, A, W, b), core_ids=list(range(NCORES)),
        trace=trace,
    )
    Y = np.concatenate(
        [np.asarray(res.results[i]["out"]).T for i in range(NCORES)], axis=0
    )
    return np.ascontiguousarray(Y, dtype=np.float32), res


def kernel(H, A, W, b):
    return run(H, A, W, b)[0]


# revision 4
# speedup vs baseline: 1.0242x; 1.0242x over previous
"""Trainium2 Bass kernel: basic GCN layer, row-parallel over 8 NeuronCores.

    Y = relu( D^-1/2 (A + I) D^-1/2 (H @ W.T + b) ),  D = (A + I).sum(axis=1)

Sharding: core i owns output rows [i*1024, (i+1)*1024).  Each core receives
(A + I)[rows, :].T pre-tiled host-side into [128, 64*1024] fp8 so every DMA
descriptor moves an 8 KiB contiguous line (A+I is 0/1/2 — fp8 is lossless).
A stays fp8 in SBUF; matmuls use it as the moving operand against bf16
stationaries (mixed-dtype matmul is legal on TRN2).

Schedule per core (the first collective cannot start before the ~50-65 us
entry barrier + ~11 us ncfw setup, so everything before it is free time):
  - A loads in 8 chunks split over the gpsimd+sync DMA queues; the PE runs
    the row-sum matmuls (ones^T @ A-tile) paced with the arriving chunks.
  - H.T follows on the same queues; HW = H @ W.T + b is computed in
    4-k-tile batches per PSUM bank, copied to SBUF by the scalar engine
    (keeps the vector queue clear and avoids per-tile ping-pong stalls).
  - Row sums go out in ONE 8-rank AllGather (4 KiB), triggered right
    after the row-sum matmuls finish.
  - When the AG lands: gathered sums -> PE transpose -> dinv = 1/sqrt on
    [128, 64]; the 64 per-k-tile scales of HW run on vector+gpsimd
    alternately, racing ahead of the main matmuls (X^T A-tile into two
    PSUM halves); epilogue relu(dl * main); DMA out per half.
"""

import os
import sys

import numpy as np

for _p in ("/opt/trn_rl_repo", "/root/.axon_site/_ro/trn_rl_repo"):
    if _p not in sys.path and os.path.isdir(_p):
        sys.path.insert(0, _p)

N = 8192        # nodes
NCORES = 8
RPC = N // NCORES  # rows per core (1024)
P = 128         # partitions / tile edge
F = 128         # feature dim (in == out)


def _build_nc(n=8192, rpc=1024, f=128, ncores=8):
    import concourse.bass as bass  # noqa: F401
    import concourse.mybir as mybir
    from concourse import bacc, tile
    from concourse.masks import make_identity

    dt = mybir.dt
    f32, bf, f8 = dt.float32, dt.bfloat16, dt.float8e4

    P = 128
    kt = n // P                 # contraction tiles (64)
    NCH = 8                     # A DMA chunks
    kpc = kt // NCH             # k-tiles per chunk (8)
    RC = 512                    # PSUM half width
    NRC = rpc // RC             # 2 halves
    KB = 4                      # k-tiles batched per hw-pre PSUM bank
    NB = kt // KB               # 16 hw-pre batches

    nc = bacc.Bacc("TRN2", num_devices=ncores)

    at = nc.dram_tensor("at", [P, kt * rpc], f8, kind="ExternalInput")   # (A+I)[rows].T pre-tiled
    ht = nc.dram_tensor("ht", [f, n], bf, kind="ExternalInput")          # H.T
    wt = nc.dram_tensor("wt", [f, f], bf, kind="ExternalInput")          # W.T
    bias = nc.dram_tensor("bias", [1, f], bf, kind="ExternalInput")      # b
    out = nc.dram_tensor("out", [f, rpc], f32, kind="ExternalOutput")    # Y[rows].T

    with tile.TileContext(nc) as tc:
        with (
            tc.tile_pool(name="const", bufs=1) as cpool,
            tc.tile_pool(name="abuf", bufs=1) as apool,
            tc.tile_pool(name="work", bufs=1) as wpool,
            tc.tile_pool(name="pshw", bufs=2, space="PSUM") as pshw,
            tc.tile_pool(name="psbig", bufs=1, space="PSUM") as psbig,
            tc.tile_pool(name="dram", bufs=1, space="DRAM") as dpool,
        ):
            # ---- constants / small inputs ----
            wt_sb = cpool.tile([f, f], bf, tag="wt", name="wt_sb")
            bias_sb = cpool.tile([1, f], bf, tag="bias", name="bias_sb")
            ones_c = cpool.tile([P, 1], bf, tag="onesc", name="ones_c")
            ones_r = cpool.tile([1, P], bf, tag="onesr", name="ones_r")
            ident = cpool.tile([P, P], f32, tag="ident", name="ident")
            nc.vector.memset(ones_c[:], 1.0)
            nc.vector.memset(ones_r[:], 1.0)
            make_identity(nc, ident[:])
            nc.scalar.dma_start(wt_sb[:], wt[:])
            nc.scalar.dma_start(bias_sb[:], bias[:])

            # ---- A chunks, then H.T chunks, on gpsimd+sync queues ----
            a_ch = []
            for c in range(NCH):
                a_c = apool.tile([P, kpc * rpc], f8, tag=f"a{c}", name=f"a{c}")
                eng = nc.gpsimd if c % 2 == 0 else nc.sync
                eng.dma_start(a_c[:], at[:, c * kpc * rpc:(c + 1) * kpc * rpc])
                a_ch.append(a_c)

            ht_ch = []
            for c in range(NCH):
                h_c = cpool.tile([f, rpc], bf, tag=f"h{c}", name=f"ht{c}")
                eng = nc.gpsimd if c % 2 == 0 else nc.sync
                eng.dma_start(h_c[:], ht[:, c * rpc:(c + 1) * rpc])
                ht_ch.append(h_c)

            def a_slice(k, h):
                return a_ch[k // kpc][:, (k % kpc) * rpc + h * RC:
                                      (k % kpc) * rpc + (h + 1) * RC]

            # ---- row sums, paced with the A chunks ----
            ps_rs = [psbig.tile([1, RC], f32, tag=f"rs{h}", name=f"rs{h}")
                     for h in range(NRC)]
            for k in range(kt):
                for h in range(NRC):
                    nc.tensor.matmul(ps_rs[h][0:1, :], ones_c[:, 0:1],
                                     a_slice(k, h),
                                     start=(k == 0), stop=(k == kt - 1))

            # row sums -> SBUF (scalar engine) -> DRAM -> AllGather
            rs_sb = wpool.tile([1, rpc], f32, tag="rs_sb", name="rs_sb")
            for h in range(NRC):
                nc.scalar.copy(rs_sb[0:1, h * RC:(h + 1) * RC],
                               ps_rs[h][0:1, :])
            ci = dpool.tile([1, rpc], f32, tag="ccin", name="cc_in")
            co = dpool.tile([ncores, rpc], f32, tag="ccout", name="cc_out",
                            addr_space="Shared")
            nc.gpsimd.dma_start(ci[:], rs_sb[:])
            nc.gpsimd.collective_compute(
                "AllGather", mybir.AluOpType.bypass,
                replica_groups=[list(range(ncores))],
                ins=[ci.opt()], outs=[co.opt()],
            )

            # ---- HW = H @ W.T + b, 4 k-tiles per PSUM bank ----
            hw_sb = wpool.tile([P, n], bf, tag="hw", name="hw_sb")
            for g in range(NB):
                ps4 = pshw.tile([P, KB * f], f32, tag="hw4", name=f"hw4_{g}")
                for m in range(KB):
                    k = g * KB + m
                    sl = ps4[:, m * f:(m + 1) * f]
                    nc.tensor.matmul(sl,
                                     ht_ch[k // kpc][:, (k % kpc) * P:
                                                     (k % kpc + 1) * P],
                                     wt_sb[:, :], start=True, stop=False)
                    nc.tensor.matmul(sl, ones_r[0:1, 0:P],
                                     bias_sb[0:1, :], start=False, stop=True)
                nc.scalar.copy(hw_sb[:, g * KB * f:(g + 1) * KB * f], ps4[:, :])

            # ---- gathered sums -> dinv[p, k] = 1/sqrt(s[128k + p]) ----
            rs2d = wpool.tile([kt, P], f32, tag="rs2d", name="rs2d")
            nc.gpsimd.dma_start(
                rs2d[:], co[:].rearrange("g (m p) -> (g m) p", p=P))
            ps_t = pshw.tile([P, kt], f32, tag="hw4", name="ps_t")
            nc.tensor.transpose(ps_t[:, :], rs2d[:, :], ident[0:kt, 0:kt])
            dinv = wpool.tile([P, kt], f32, tag="dinv", name="dinv")
            nc.scalar.sqrt(dinv[:, :], ps_t[:, :])
            nc.vector.reciprocal(dinv[:, :], dinv[:, :])

            # ---- dl broadcast for the epilogue: dlb[p, i] = D_i^-1/2 ----
            dlb = wpool.tile([P, rpc], f32, tag="dlb", name="dlb")
            nc.sync.dma_start(
                dlb[:].rearrange("p (o r) -> p o r", o=1),
                ci[0:1, :].partition_broadcast(P),
            )
            nc.scalar.sqrt(dlb[:, :], dlb[:, :])
            nc.vector.reciprocal(dlb[:, :], dlb[:, :])

            # ---- scale + main matmuls: ps_main[h] += (dinv_k*HW_k)^T A_k ----
            ps_main = [psbig.tile([f, RC], f32, tag=f"main{h}", name=f"main{h}")
                       for h in range(NRC)]
            for k in range(kt):
                sl = hw_sb[:, k * f:(k + 1) * f]
                eng = nc.vector if k % 2 == 0 else nc.gpsimd
                eng.tensor_scalar_mul(sl, sl, dinv[:, k:k + 1])
                for h in range(NRC):
                    nc.tensor.matmul(
                        ps_main[h][:, :], sl, a_slice(k, h),
                        start=(k == 0), stop=(k == kt - 1),
                    )

            # ---- epilogue: Y.T = relu(dl * main), DMA out per half ----
            y_sb = wpool.tile([f, rpc], f32, tag="y", name="y_sb")
            for h in range(NRC):
                sl = y_sb[:, h * RC:(h + 1) * RC]
                nc.vector.tensor_mul(sl, ps_main[h][:, :],
                                     dlb[:, h * RC:(h + 1) * RC])
                nc.vector.tensor_scalar_max(sl, sl, 0.0)
                nc.gpsimd.dma_start(out[:, h * RC:(h + 1) * RC], sl)

    nc.compile()
    return nc


_CACHE = {}


def _get_nc():
    if "nc" not in _CACHE:
        _CACHE["nc"] = _build_nc()
    return _CACHE["nc"]


def _prep_in_maps(H, A, W, b):
    import ml_dtypes

    bf16 = ml_dtypes.bfloat16
    H = np.asarray(H, dtype=np.float32)
    A = np.asarray(A, dtype=np.float32)
    W = np.asarray(W, dtype=np.float32)
    b = np.asarray(b, dtype=np.float32)
    ht = np.ascontiguousarray(H.T.astype(bf16))
    wt = np.ascontiguousarray(W.T.astype(bf16))
    bias = np.ascontiguousarray(b.reshape(1, -1).astype(bf16))
    idx = np.arange(RPC)
    maps = []
    for i in range(NCORES):
        rows = slice(i * RPC, (i + 1) * RPC)
        Asl = A[rows, :].copy()
        Asl[idx, i * RPC + idx] += 1.0          # fold in A + I (0/1/2: exact)
        # pre-tile (A+I)[rows].T -> [128, kt*rpc] with 8 KiB-contiguous lines
        at = Asl.T.reshape(N // P, P, RPC).transpose(1, 0, 2).reshape(P, -1)
        maps.append({
            "at": np.ascontiguousarray(at.astype(ml_dtypes.float8_e4m3)),
            "ht": ht,
            "wt": wt,
            "bias": bias,
        })
    return maps


def run(H, A, W, b, trace=False):
    from concourse import bass_utils

    nc = _get_nc()
    res = bass_utils.run_bass_kernel_spmd(
        nc, _prep_in_maps(H, A, W, b), core_ids=list(range(NCORES)),
        trace=trace,
    )
    Y = np.concatenate(
        [np.asarray(res.results[i]["out"]).T for i in range(NCORES)], axis=0
    )
    return np.ascontiguousarray(Y, dtype=np.float32), res


def kernel(H, A, W, b):
    return run(H, A, W, b)[0]


# revision 8
# speedup vs baseline: 1.1686x; 1.1410x over previous
"""Trainium2 Bass kernel: basic GCN layer, row-parallel over 8 NeuronCores.

    Y = relu( D^-1/2 (A + I) D^-1/2 (H @ W.T + b) ),  D = (A + I).sum(axis=1)

Sharding: core i owns output rows [i*1024, (i+1)*1024).  Each core receives
(A + I)[rows, :].T pre-tiled host-side into [128, 64*1024] fp8 so every DMA
descriptor moves an 8 KiB contiguous line (A+I is 0/1/2 — fp8 is lossless).
A stays fp8 in SBUF; matmuls use it as the moving operand against bf16
stationaries (mixed-dtype matmul is legal on TRN2).

Schedule per core (the first collective cannot start before the ~50-65 us
entry barrier + ~11 us ncfw setup, so everything before it is free time):
  - A loads in 8 chunks split over the gpsimd+sync DMA queues; the PE runs
    the row-sum matmuls (ones^T @ A-tile) paced with the arriving chunks.
  - H.T follows on the same queues; HW = H @ W.T + b is computed in
    4-k-tile batches per PSUM bank, copied to SBUF by the scalar engine
    (keeps the vector queue clear and avoids per-tile ping-pong stalls).
  - Row sums go out in ONE 8-rank AllGather (4 KiB), triggered right
    after the row-sum matmuls finish.
  - When the AG lands: gathered sums -> PE transpose -> dinv = 1/sqrt on
    [128, 64]; the 64 per-k-tile scales of HW run on vector+gpsimd
    alternately, racing ahead of the main matmuls (X^T A-tile into two
    PSUM halves); epilogue relu(dl * main); DMA out per half.
"""

import os
import sys

import numpy as np

for _p in ("/opt/trn_rl_repo", "/root/.axon_site/_ro/trn_rl_repo"):
    if _p not in sys.path and os.path.isdir(_p):
        sys.path.insert(0, _p)

N = 8192        # nodes
NCORES = 8
RPC = N // NCORES  # rows per core (1024)
P = 128         # partitions / tile edge
F = 128         # feature dim (in == out)


def _build_nc(n=8192, rpc=1024, f=128, ncores=8):
    import concourse.bass as bass  # noqa: F401
    import concourse.mybir as mybir
    from concourse import bacc, tile
    from concourse.masks import make_identity

    dt = mybir.dt
    f32, bf, f8 = dt.float32, dt.bfloat16, dt.float8e4

    P = 128
    kt = n // P                 # contraction tiles (64)
    NCH = 8                     # A DMA chunks
    kpc = kt // NCH             # k-tiles per chunk (8)
    RC = 512                    # PSUM half width
    NRC = rpc // RC             # 2 halves
    KB = 4                      # k-tiles batched per hw-pre PSUM bank
    NB = kt // KB               # 16 hw-pre batches

    nc = bacc.Bacc("TRN2", num_devices=ncores)

    at = nc.dram_tensor("at", [P, kt * rpc], f8, kind="ExternalInput")   # (A+I)[rows].T pre-tiled
    ht = nc.dram_tensor("ht", [f, n], bf, kind="ExternalInput")          # H.T
    wt = nc.dram_tensor("wt", [f, f], bf, kind="ExternalInput")          # W.T
    bias = nc.dram_tensor("bias", [1, f], bf, kind="ExternalInput")      # b
    out = nc.dram_tensor("out", [f, rpc], f32, kind="ExternalOutput")    # Y[rows].T

    with tile.TileContext(nc) as tc:
        with (
            tc.tile_pool(name="const", bufs=1) as cpool,
            tc.tile_pool(name="abuf", bufs=1) as apool,
            tc.tile_pool(name="work", bufs=1) as wpool,
            tc.tile_pool(name="pshw", bufs=3, space="PSUM") as pshw,
            tc.tile_pool(name="psbig", bufs=1, space="PSUM") as psbig,
            tc.tile_pool(name="dram", bufs=1, space="DRAM") as dpool,
        ):
            # ---- constants / small inputs ----
            wt_sb = cpool.tile([f, f], bf, tag="wt", name="wt_sb")
            bias_sb = cpool.tile([1, f], bf, tag="bias", name="bias_sb")
            ones_c = cpool.tile([P, 1], bf, tag="onesc", name="ones_c")
            ones_r = cpool.tile([1, P], bf, tag="onesr", name="ones_r")
            ident = cpool.tile([P, P], f32, tag="ident", name="ident")
            nc.vector.memset(ones_c[:], 1.0)
            nc.vector.memset(ones_r[:], 1.0)
            make_identity(nc, ident[:])
            nc.scalar.dma_start(wt_sb[:], wt[:])
            nc.scalar.dma_start(bias_sb[:], bias[:])

            # ---- A over three DMA queues; first H.T chunks early ----
            a_ch = [apool.tile([P, kpc * rpc], f8, tag=f"a{c}", name=f"a{c}")
                    for c in range(NCH)]
            ht_ch = [cpool.tile([f, rpc], bf, tag=f"h{c}", name=f"ht{c}")
                     for c in range(NCH)]

            def a_dma(eng, c):
                eng.dma_start(a_ch[c][:],
                              at[:, c * kpc * rpc:(c + 1) * kpc * rpc])

            def ht_dma(eng, c):
                eng.dma_start(ht_ch[c][:], ht[:, c * rpc:(c + 1) * rpc])

            ht_dma(nc.gpsimd, 0)
            ht_dma(nc.sync, 1)
            for c in (0, 2, 4):
                a_dma(nc.gpsimd, c)
            for c in (1, 3, 5):
                a_dma(nc.sync, c)
            a_dma(nc.scalar, 6)
            a_dma(nc.scalar, 7)
            for c in (2, 4, 6):
                ht_dma(nc.gpsimd, c)
            for c in (3, 5, 7):
                ht_dma(nc.sync, c)

            def a_slice(k, h):
                return a_ch[k // kpc][:, (k % kpc) * rpc + h * RC:
                                      (k % kpc) * rpc + (h + 1) * RC]

            # ---- row sums, paced with the A chunks ----
            ps_rs = [psbig.tile([1, RC], f32, tag=f"rs{h}", name=f"rs{h}")
                     for h in range(NRC)]
            for k in range(kt):
                for h in range(NRC):
                    nc.tensor.matmul(ps_rs[h][0:1, :], ones_c[:, 0:1],
                                     a_slice(k, h),
                                     start=(k == 0), stop=(k == kt - 1))

            # row sums -> SBUF (vector engine) -> DRAM -> AllGather
            rs_sb = wpool.tile([1, rpc], f32, tag="rs_sb", name="rs_sb")
            for h in range(NRC):
                nc.vector.tensor_copy(rs_sb[0:1, h * RC:(h + 1) * RC],
                                      ps_rs[h][0:1, :])
            ci = dpool.tile([1, rpc], f32, tag="ccin", name="cc_in")
            co = dpool.tile([ncores, rpc], f32, tag="ccout", name="cc_out",
                            addr_space="Shared")
            nc.gpsimd.dma_start(ci[:], rs_sb[:])
            nc.gpsimd.collective_compute(
                "AllGather", mybir.AluOpType.bypass,
                replica_groups=[list(range(ncores))],
                ins=[ci.opt()], outs=[co.opt()],
            )

            # ---- HW = H @ W.T + b, 4 k-tiles per PSUM bank ----
            hw_sb = wpool.tile([P, n], bf, tag="hw", name="hw_sb")
            for g in range(NB):
                ps4 = pshw.tile([P, KB * f], f32, tag="hw4", name=f"hw4_{g}")
                for m in range(KB):
                    k = g * KB + m
                    sl = ps4[:, m * f:(m + 1) * f]
                    nc.tensor.matmul(sl,
                                     ht_ch[k // kpc][:, (k % kpc) * P:
                                                     (k % kpc + 1) * P],
                                     wt_sb[:, :], start=True, stop=False)
                    nc.tensor.matmul(sl, ones_r[0:1, 0:P],
                                     bias_sb[0:1, :], start=False, stop=True)
                nc.scalar.copy(hw_sb[:, g * KB * f:(g + 1) * KB * f], ps4[:, :])

            # ---- gathered sums -> dinv[p, k] = 1/sqrt(s[128k + p]) ----
            rs2d = wpool.tile([kt, P], f32, tag="rs2d", name="rs2d")
            nc.gpsimd.dma_start(
                rs2d[:], co[:].rearrange("g (m p) -> (g m) p", p=P))
            ps_t = pshw.tile([P, kt], f32, tag="hw4", name="ps_t")
            nc.tensor.transpose(ps_t[:, :], rs2d[:, :], ident[0:kt, 0:kt])
            dinv = wpool.tile([P, kt], f32, tag="dinv", name="dinv")
            nc.scalar.sqrt(dinv[:, :], ps_t[:, :])
            nc.vector.reciprocal(dinv[:, :], dinv[:, :])

            # ---- dl broadcast for the epilogue: dlb[p, i] = D_i^-1/2 ----
            dlb = wpool.tile([P, rpc], f32, tag="dlb", name="dlb")
            nc.sync.dma_start(
                dlb[:].rearrange("p (o r) -> p o r", o=1),
                ci[0:1, :].partition_broadcast(P),
            )

            # ---- scale + main matmuls: ps_main[h] += (dinv_k*HW_k)^T A_k ----
            # scales alternate vector/scalar so either engine only has to
            # keep half pace with the PE; dlb's rsqrt is slipped in early.
            ps_main = [psbig.tile([f, RC], f32, tag=f"main{h}", name=f"main{h}")
                       for h in range(NRC)]
            for k in range(kt):
                sl = hw_sb[:, k * f:(k + 1) * f]
                if k % 2 == 0:
                    nc.vector.tensor_scalar_mul(sl, sl, dinv[:, k:k + 1])
                else:
                    nc.scalar.mul(sl, sl, dinv[:, k:k + 1])
                for h in range(NRC):
                    nc.tensor.matmul(
                        ps_main[h][:, :], sl, a_slice(k, h),
                        start=(k == 0), stop=(k == kt - 1),
                    )
                if k == 7:
                    nc.scalar.sqrt(dlb[:, :], dlb[:, :])
                    nc.vector.reciprocal(dlb[:, :], dlb[:, :])

            # ---- epilogue: Y.T = relu(dl * main), DMA out per half ----
            y_sb = wpool.tile([f, rpc], f32, tag="y", name="y_sb")
            for h in range(NRC):
                sl = y_sb[:, h * RC:(h + 1) * RC]
                nc.vector.tensor_mul(sl, ps_main[h][:, :],
                                     dlb[:, h * RC:(h + 1) * RC])
                nc.vector.tensor_scalar_max(sl, sl, 0.0)
                nc.gpsimd.dma_start(out[:, h * RC:(h + 1) * RC], sl)

    nc.compile()
    return nc


_CACHE = {}


def _get_nc():
    if "nc" not in _CACHE:
        _CACHE["nc"] = _build_nc()
    return _CACHE["nc"]


def _prep_in_maps(H, A, W, b):
    import ml_dtypes

    bf16 = ml_dtypes.bfloat16
    H = np.asarray(H, dtype=np.float32)
    A = np.asarray(A, dtype=np.float32)
    W = np.asarray(W, dtype=np.float32)
    b = np.asarray(b, dtype=np.float32)
    ht = np.ascontiguousarray(H.T.astype(bf16))
    wt = np.ascontiguousarray(W.T.astype(bf16))
    bias = np.ascontiguousarray(b.reshape(1, -1).astype(bf16))
    idx = np.arange(RPC)
    maps = []
    for i in range(NCORES):
        rows = slice(i * RPC, (i + 1) * RPC)
        Asl = A[rows, :].copy()
        Asl[idx, i * RPC + idx] += 1.0          # fold in A + I (0/1/2: exact)
        # pre-tile (A+I)[rows].T -> [128, kt*rpc] with 8 KiB-contiguous lines
        at = Asl.T.reshape(N // P, P, RPC).transpose(1, 0, 2).reshape(P, -1)
        maps.append({
            "at": np.ascontiguousarray(at.astype(ml_dtypes.float8_e4m3)),
            "ht": ht,
            "wt": wt,
            "bias": bias,
        })
    return maps


def run(H, A, W, b, trace=False):
    from concourse import bass_utils

    nc = _get_nc()
    res = bass_utils.run_bass_kernel_spmd(
        nc, _prep_in_maps(H, A, W, b), core_ids=list(range(NCORES)),
        trace=trace,
    )
    Y = np.concatenate(
        [np.asarray(res.results[i]["out"]).T for i in range(NCORES)], axis=0
    )
    return np.ascontiguousarray(Y, dtype=np.float32), res


def kernel(H, A, W, b):
    return run(H, A, W, b)[0]


# revision 18
# speedup vs baseline: 1.2039x; 1.0302x over previous
"""Trainium2 Bass kernel: basic GCN layer, row-parallel over 8 NeuronCores.

    Y = relu( D^-1/2 (A + I) D^-1/2 (H @ W.T + b) ),  D = (A + I).sum(axis=1)

Sharding: core i owns output rows [i*1024, (i+1)*1024).  Each core receives
(A + I)[rows, :].T pre-tiled host-side into [128, 64*1024] fp8 so every DMA
descriptor moves an 8 KiB contiguous line (A+I is 0/1/2 — fp8 is lossless).
A stays fp8 in SBUF; matmuls use it as the moving operand against bf16
stationaries (mixed-dtype matmul is legal on TRN2).

Schedule per core (the first collective cannot start before the ~50-65 us
entry barrier + ~11 us ncfw setup, so everything before it is free time):
  - A loads in 8 chunks split over the gpsimd+sync DMA queues; the PE runs
    the row-sum matmuls (ones^T @ A-tile) paced with the arriving chunks.
  - H.T follows on the same queues; HW = H @ W.T + b is computed in
    4-k-tile batches per PSUM bank, copied to SBUF by the scalar engine
    (keeps the vector queue clear and avoids per-tile ping-pong stalls).
  - Row sums go out in ONE 8-rank AllGather (4 KiB), triggered right
    after the row-sum matmuls finish.
  - When the AG lands: gathered sums -> PE transpose -> dinv = 1/sqrt on
    [128, 64]; the 64 per-k-tile scales of HW run on vector+gpsimd
    alternately, racing ahead of the main matmuls (X^T A-tile into two
    PSUM halves); epilogue relu(dl * main); DMA out per half.
"""

import os
import sys

import numpy as np

for _p in ("/opt/trn_rl_repo", "/root/.axon_site/_ro/trn_rl_repo"):
    if _p not in sys.path and os.path.isdir(_p):
        sys.path.insert(0, _p)

N = 8192        # nodes
NCORES = 8
RPC = N // NCORES  # rows per core (1024)
P = 128         # partitions / tile edge
F = 128         # feature dim (in == out)


def _build_nc(n=8192, rpc=1024, f=128, ncores=8):
    import concourse.bass as bass  # noqa: F401
    import concourse.mybir as mybir
    from concourse import bacc, tile
    from concourse.masks import make_identity

    dt = mybir.dt
    f32, bf, f8 = dt.float32, dt.bfloat16, dt.float8e4

    P = 128
    kt = n // P                 # contraction tiles (64)
    NCH = 8                     # A DMA chunks
    kpc = kt // NCH             # k-tiles per chunk (8)
    RC = 512                    # PSUM half width
    NRC = rpc // RC             # 2 halves
    KB = 4                      # k-tiles batched per hw-pre PSUM bank
    NB = kt // KB               # 16 hw-pre batches

    nc = bacc.Bacc("TRN2", num_devices=ncores)

    at = nc.dram_tensor("at", [P, kt * rpc], f8, kind="ExternalInput")   # (A+I)[rows].T pre-tiled
    ht = nc.dram_tensor("ht", [f, n], bf, kind="ExternalInput")          # H.T
    wt = nc.dram_tensor("wt", [f, f], bf, kind="ExternalInput")          # W.T
    bias = nc.dram_tensor("bias", [1, f], bf, kind="ExternalInput")      # b
    out = nc.dram_tensor("out", [f, rpc], f32, kind="ExternalOutput")    # Y[rows].T

    with tile.TileContext(nc) as tc:
        with (
            tc.tile_pool(name="const", bufs=1) as cpool,
            tc.tile_pool(name="abuf", bufs=1) as apool,
            tc.tile_pool(name="work", bufs=1) as wpool,
            tc.tile_pool(name="pshw", bufs=3, space="PSUM") as pshw,
            tc.tile_pool(name="psbig", bufs=1, space="PSUM") as psbig,
            tc.tile_pool(name="dram", bufs=1, space="DRAM") as dpool,
        ):
            # ---- constants / small inputs ----
            wt_sb = cpool.tile([f, f], bf, tag="wt", name="wt_sb")
            bias_sb = cpool.tile([1, f], bf, tag="bias", name="bias_sb")
            ones_c = cpool.tile([P, 1], bf, tag="onesc", name="ones_c")
            ones_r = cpool.tile([1, P], bf, tag="onesr", name="ones_r")
            ident = cpool.tile([P, P], f32, tag="ident", name="ident")
            nc.vector.memset(ones_c[:], 1.0)
            nc.vector.memset(ones_r[:], 1.0)
            make_identity(nc, ident[:])
            nc.scalar.dma_start(wt_sb[:], wt[:])
            nc.scalar.dma_start(bias_sb[:], bias[:])

            # ---- A balanced over four DMA rings, H.T trailing ----
            a_ch = [apool.tile([P, kpc * rpc], f8, tag=f"a{c}", name=f"a{c}")
                    for c in range(NCH)]
            ht_ch = [cpool.tile([f, rpc], bf, tag=f"h{c}", name=f"ht{c}")
                     for c in range(NCH)]

            def a_dma(eng, c, parts=1):
                w = kpc * rpc
                for q in range(parts):
                    eng.dma_start(
                        a_ch[c][:, q * w // parts:(q + 1) * w // parts],
                        at[:, c * w + q * w // parts:
                           c * w + (q + 1) * w // parts])

            def ht_dma(eng, c):
                eng.dma_start(ht_ch[c][:], ht[:, c * rpc:(c + 1) * rpc])

            ring_of = [nc.gpsimd, nc.sync, nc.scalar, nc.gpsimd,
                       nc.sync, nc.scalar, nc.gpsimd, nc.sync]
            for c in range(NCH):
                a_dma(ring_of[c], c, parts=2 if c < 3 else 1)
            for c in range(NCH):
                ht_dma(ring_of[c], c)

            def a_slice(k, h):
                return a_ch[k // kpc][:, (k % kpc) * rpc + h * RC:
                                      (k % kpc) * rpc + (h + 1) * RC]

            # ---- row sums, paced with the A chunks ----
            ps_rs = [psbig.tile([1, RC], f32, tag=f"rs{h}", name=f"rs{h}")
                     for h in range(NRC)]
            for k in range(kt):
                for h in range(NRC):
                    nc.tensor.matmul(ps_rs[h][0:1, :], ones_c[:, 0:1],
                                     a_slice(k, h),
                                     start=(k == 0), stop=(k == kt - 1))

            # row sums -> SBUF (vector engine) -> DRAM -> AllGather
            rs_sb = wpool.tile([1, rpc], f32, tag="rs_sb", name="rs_sb")
            for h in range(NRC):
                nc.vector.tensor_copy(rs_sb[0:1, h * RC:(h + 1) * RC],
                                      ps_rs[h][0:1, :])
            ci = dpool.tile([1, rpc], f32, tag="ccin", name="cc_in")
            co = dpool.tile([ncores, rpc], f32, tag="ccout", name="cc_out",
                            addr_space="Shared")
            nc.gpsimd.dma_start(ci[:], rs_sb[:])
            nc.gpsimd.collective_compute(
                "AllGather", mybir.AluOpType.bypass,
                replica_groups=[list(range(ncores))],
                ins=[ci.opt()], outs=[co.opt()],
            )

            # ---- HW = H @ W.T + b, 4 k-tiles per PSUM bank ----
            hw_sb = wpool.tile([P, n], bf, tag="hw", name="hw_sb")
            for g in range(NB):
                ps4 = pshw.tile([P, KB * f], f32, tag="hw4", name=f"hw4_{g}")
                for m in range(KB):
                    k = g * KB + m
                    sl = ps4[:, m * f:(m + 1) * f]
                    nc.tensor.matmul(sl,
                                     ht_ch[k // kpc][:, (k % kpc) * P:
                                                     (k % kpc + 1) * P],
                                     wt_sb[:, :], start=True, stop=False)
                    nc.tensor.matmul(sl, ones_r[0:1, 0:P],
                                     bias_sb[0:1, :], start=False, stop=True)
                nc.scalar.copy(hw_sb[:, g * KB * f:(g + 1) * KB * f], ps4[:, :])

            # ---- PE warm-keeper toward the AG landing ----
            warm_src = cpool.tile([1, 512], bf, tag="warm", name="warm_src")
            nc.vector.memset(warm_src[:], 1.0)
            ps_warm = pshw.tile([1, 512], f32, tag="hw4", name="ps_warm")
            for _ in range(36):
                nc.tensor.matmul(ps_warm[0:1, :], warm_src[0:1, 0:1],
                                 warm_src[0:1, :], start=True, stop=True)

            # ---- gathered sums -> dinv[p, k] = 1/sqrt(s[128k + p]) ----
            rs2d = wpool.tile([kt, P], f32, tag="rs2d", name="rs2d")
            nc.sync.dma_start(
                rs2d[:], co[:].rearrange("g (m p) -> (g m) p", p=P))
            ps_t = pshw.tile([P, kt], f32, tag="hw4", name="ps_t")
            nc.tensor.transpose(ps_t[:, :], rs2d[:, :], ident[0:kt, 0:kt])
            dinv = wpool.tile([P, kt], f32, tag="dinv", name="dinv")
            nc.scalar.sqrt(dinv[:, :], ps_t[:, :])
            nc.vector.reciprocal(dinv[:, :], dinv[:, :])

            # ---- dl broadcast for the epilogue: dlb[p, i] = D_i^-1/2 ----
            dlb = wpool.tile([P, rpc], f32, tag="dlb", name="dlb")
            nc.sync.dma_start(
                dlb[:].rearrange("p (o r) -> p o r", o=1),
                ci[0:1, :].partition_broadcast(P),
            )

            # ---- scale + main matmuls: ps_main[h] += (dinv_k*HW_k)^T A_k ----
            # scales alternate vector/scalar so either engine only has to
            # keep half pace with the PE; dlb's rsqrt is slipped in early.
            ps_main = [psbig.tile([f, RC], f32, tag=f"main{h}", name=f"main{h}")
                       for h in range(NRC)]
            for k in range(kt):
                sl = hw_sb[:, k * f:(k + 1) * f]
                if k % 2 == 0:
                    nc.vector.tensor_scalar_mul(sl, sl, dinv[:, k:k + 1])
                else:
                    nc.scalar.mul(sl, sl, dinv[:, k:k + 1])
                for h in range(NRC):
                    nc.tensor.matmul(
                        ps_main[h][:, :], sl, a_slice(k, h),
                        start=(k == 0), stop=(k == kt - 1),
                    )
                if k == 7:
                    nc.scalar.sqrt(dlb[:, :], dlb[:, :])
                    nc.vector.reciprocal(dlb[:, :], dlb[:, :])

            # ---- epilogue: Y.T = relu(dl * main), DMA out per half ----
            y_sb = wpool.tile([f, rpc], f32, tag="y", name="y_sb")
            for h in range(NRC):
                sl = y_sb[:, h * RC:(h + 1) * RC]
                nc.vector.tensor_mul(sl, ps_main[h][:, :],
                                     dlb[:, h * RC:(h + 1) * RC])
                nc.vector.tensor_scalar_max(sl, sl, 0.0)
                eng = nc.gpsimd if h == 0 else nc.sync
                eng.dma_start(out[:, h * RC:(h + 1) * RC], sl)

    nc.compile()
    return nc


_CACHE = {}


def _get_nc():
    if "nc" not in _CACHE:
        _CACHE["nc"] = _build_nc()
    return _CACHE["nc"]


def _prep_in_maps(H, A, W, b):
    import ml_dtypes

    bf16 = ml_dtypes.bfloat16
    H = np.asarray(H, dtype=np.float32)
    A = np.asarray(A, dtype=np.float32)
    W = np.asarray(W, dtype=np.float32)
    b = np.asarray(b, dtype=np.float32)
    ht = np.ascontiguousarray(H.T.astype(bf16))
    wt = np.ascontiguousarray(W.T.astype(bf16))
    bias = np.ascontiguousarray(b.reshape(1, -1).astype(bf16))
    idx = np.arange(RPC)
    maps = []
    for i in range(NCORES):
        rows = slice(i * RPC, (i + 1) * RPC)
        Asl = A[rows, :].copy()
        Asl[idx, i * RPC + idx] += 1.0          # fold in A + I (0/1/2: exact)
        # pre-tile (A+I)[rows].T -> [128, kt*rpc] with 8 KiB-contiguous lines
        at = Asl.T.reshape(N // P, P, RPC).transpose(1, 0, 2).reshape(P, -1)
        maps.append({
            "at": np.ascontiguousarray(at.astype(ml_dtypes.float8_e4m3)),
            "ht": ht,
            "wt": wt,
            "bias": bias,
        })
    return maps


def run(H, A, W, b, trace=False):
    from concourse import bass_utils

    nc = _get_nc()
    res = bass_utils.run_bass_kernel_spmd(
        nc, _prep_in_maps(H, A, W, b), core_ids=list(range(NCORES)),
        trace=trace,
    )
    Y = np.concatenate(
        [np.asarray(res.results[i]["out"]).T for i in range(NCORES)], axis=0
    )
    return np.ascontiguousarray(Y, dtype=np.float32), res


def kernel(H, A, W, b):
    return run(H, A, W, b)[0]
